# revision 9
# baseline (speedup 1.0000x reference)
"""RNN-T JointNet fused Bass kernel for Trainium2, SPMD over 8 NeuronCores.

Reference computation (all fp32):
    enc = LN(encoder_out @ W_enc + b_enc) * g_enc + be_enc      # [B,T,J]
    dec = LN(decoder_out @ W_dec + b_dec) * g_dec + be_dec      # [B,U,J]
    joint = relu(enc[:,:,None,:] + dec[:,None,:,:])             # [B,T,U,J]
    out = joint @ W_out + b_out                                 # [B,T,U,V]

Shapes: B=4, T=512, U=64, E=D=J=512, V=1024.

Sharding: data-parallel over the flattened (B,T) axis. Core c owns
b = c//2, t in [(c%2)*256, (c%2)*256+256) -> 16384 output rows, which are
contiguous in the flattened [B*T*U, V] output, so the gather is a concat.

Design notes:
  - PE column clock is 1 col/cycle @2.4GHz for every dtype >= bf16 (fp8
    DoubleRow only doubles contraction, which accuracy splits cancel), so
    the main GEMM floor is 1024 matmuls x ~216ns = 221us/core. All other
    work is sized/placed to hide under that.
  - bf16 end-to-end: host pre-casts + pre-transposes inputs (no phase-A
    PE transposes), bf16 weights/joint/jr halve SBUF traffic, output is
    written bf16 (~101us DMA, hidden) and upcast on host.
  - Input DMAs priority-ordered on one queue: dec side first (its
    projection+LN chain gates the first joint), wout last.
  - PE warmup matmuls during the DMA wait raise the p-state so real work
    runs at full clock; more warmups cover the LN latency.
  - encT/decT live in PSUM (written by PE transposes, read directly by
    DVE joint adds) - no PSUM->SBUF copies on the critical path.
  - enc tb1 projection/LN/transposes are deferred until after supertile 0
    so the first supertile starts ~7us earlier; they hide in the loop.
  - Main loop per supertile: joint add halves (DVE), relu halves (ACT),
    32 matmuls (PE), 4 evictions split DVE/ACT, 4 output DMAs.
"""

import numpy as np

B, T, U = 4, 512, 64
E = D = J = 512
V = 1024
EPS = 1e-5
P = 128
NCORES = 8
TC = T * B // NCORES            # 256 t-rows per core
ROWS = TC * U                   # 16384 output rows per core
MM_TILES = ROWS // 512          # 32 supertiles of 512 rows (8 t values)

_CACHE = {}


def _build(apply_b_enc, apply_g_enc, apply_be_enc,
           apply_b_dec, apply_g_dec, apply_be_dec, apply_b_out):
    import concourse.bass as bass
    import concourse.mybir as mybir
    import concourse.tile as tile
    from concourse import bacc
    from concourse.masks import make_identity

    f32 = mybir.dt.float32
    bf16 = mybir.dt.bfloat16
    AF = mybir.ActivationFunctionType
    OP = mybir.AluOpType

    nc = bacc.Bacc(target_bir_lowering=False)

    enc_xT = nc.dram_tensor("enc_xT", [E, TC], bf16, kind="ExternalInput")
    dec_xT = nc.dram_tensor("dec_xT", [D, U], bf16, kind="ExternalInput")
    w_enc = nc.dram_tensor("w_enc", [E, J], bf16, kind="ExternalInput")
    w_dec = nc.dram_tensor("w_dec", [D, J], bf16, kind="ExternalInput")
    w_out = nc.dram_tensor("w_out", [J, V], bf16, kind="ExternalInput")
    b_enc = nc.dram_tensor("b_enc", [J], f32, kind="ExternalInput")
    g_enc = nc.dram_tensor("g_enc", [J], f32, kind="ExternalInput")
    be_enc = nc.dram_tensor("be_enc", [J], f32, kind="ExternalInput")
    b_dec = nc.dram_tensor("b_dec", [J], f32, kind="ExternalInput")
    g_dec = nc.dram_tensor("g_dec", [J], f32, kind="ExternalInput")
    be_dec = nc.dram_tensor("be_dec", [J], f32, kind="ExternalInput")
    b_out = nc.dram_tensor("b_out", [V], f32, kind="ExternalInput")
    out = nc.dram_tensor("out", [ROWS, V], bf16, kind="ExternalOutput")

    def bcast_row(dram_vec, n):
        return bass.AP(tensor=dram_vec.tensor, offset=dram_vec.offset,
                       ap=[[0, P], [1, n]])

    from contextlib import ExitStack

    with tile.TileContext(nc) as tc, ExitStack() as ctx:
        const = ctx.enter_context(tc.tile_pool(name="const", bufs=1))
        prep = ctx.enter_context(tc.tile_pool(name="prep", bufs=2))
        jpool = ctx.enter_context(tc.tile_pool(name="jpool", bufs=2))
        jrpool = ctx.enter_context(tc.tile_pool(name="jrpool", bufs=3))
        opool = ctx.enter_context(tc.tile_pool(name="opool", bufs=6))
        mpsum = ctx.enter_context(tc.tile_pool(name="mpsum", bufs=3, space="PSUM"))
        psc = ctx.enter_context(tc.tile_pool(name="psc", bufs=1, space="PSUM"))

        # ---- input DMAs, priority-ordered on the SP queue (transfers are
        # serial per queue => earlier ones get full HBM bandwidth) ----
        dxT_sb = prep.tile([P, D // P, U], bf16, tag="dxT_sb")
        nc.sync.dma_start(dxT_sb[:], dec_xT[:].rearrange("(o p) u -> p o u", p=P))
        wdec_sb = const.tile([P, D // P, J], bf16)
        nc.sync.dma_start(wdec_sb[:], w_dec[:].rearrange("(o p) j -> p o j", p=P))
        xT_sb = prep.tile([P, E // P, TC], bf16, tag="xT_sb")
        nc.sync.dma_start(xT_sb[:], enc_xT[:].rearrange("(o p) t -> p o t", p=P))
        wenc_sb = const.tile([P, E // P, J], bf16)
        nc.sync.dma_start(wenc_sb[:], w_enc[:].rearrange("(o p) j -> p o j", p=P))
        wout_sb = const.tile([P, J // P, V], bf16)
        nc.sync.dma_start(wout_sb[:], w_out[:].rearrange("(o p) v -> p o v", p=P))

        ident = const.tile([P, P], bf16)
        make_identity(nc, ident)

        # PE warmup on a zeroed tile: raises the p-state while DMAs stream
        warm_src = const.tile([P, 512], bf16)
        nc.vector.memset(warm_src[:], 0.0)

        def warmup(n, label):
            for i in range(n):
                wps = mpsum.tile([P, 512], f32, tag="mps", name=f"w{label}_{i}")
                nc.tensor.matmul(wps, warm_src[:, :P], warm_src[:],
                                 start=True, stop=True)

        eps_sb = const.tile([P, 1], f32)
        nc.vector.memset(eps_sb[:], EPS)

        def load_vec(vec, n, enabled):
            if not enabled:
                return None
            t = const.tile([P, n], f32)
            nc.gpsimd.dma_start(t[:], bcast_row(vec, n))
            return t

        b_enc_sb = load_vec(b_enc, J, apply_b_enc)
        g_enc_sb = load_vec(g_enc, J, apply_g_enc)
        be_enc_sb = load_vec(be_enc, J, apply_be_enc)
        b_dec_sb = load_vec(b_dec, J, apply_b_dec)
        g_dec_sb = load_vec(g_dec, J, apply_g_dec)
        be_dec_sb = load_vec(be_dec, J, apply_be_dec)
        b_out_sb = load_vec(b_out, V, apply_b_out)

        # persistent PSUM tiles holding the J-major normalized activations
        encPT = psc.tile([P, J // P, TC], bf16)
        decPT = psc.tile([P, J // P, U], bf16)

        def layer_norm_psum(ps, rows, label, b_sb, g_sb, be_sb, hot=False):
            """LN over the free dim of psum tile ps [rows, J] -> bf16 SBUF."""
            ln16 = prep.tile([P, J], bf16, tag="ln16", name=f"ln16_{label}")
            if b_sb is not None:
                xf = prep.tile([P, J], f32, tag="lnf", name=f"lnf_{label}")
                nc.vector.tensor_add(xf[:rows], ps[:rows], b_sb[:rows])
                src = xf
            else:
                src = ps
            hp = tc.high_priority if hot else _null_ctx
            with hp():
                stats = prep.tile([P, 6], f32, tag="ln_stats", name=f"st_{label}")
                mv = prep.tile([P, 2], f32, tag="ln_mv", name=f"mv_{label}")
                nc.vector.bn_stats(out=stats[:rows], in_=src[:rows])
                nc.vector.bn_aggr(out=mv[:rows], in_=stats[:rows])
                rstd = prep.tile([P, 1], f32, tag="ln_rstd", name=f"rs_{label}")
                nc.scalar.activation(out=rstd[:rows], in_=mv[:rows, 1:2],
                                     func=AF.Sqrt, bias=eps_sb[:rows], scale=1.0)
                nc.vector.reciprocal(out=rstd[:rows], in_=rstd[:rows])
                nc.vector.tensor_scalar(ln16[:rows], src[:rows],
                                        mv[:rows, 0:1], rstd[:rows],
                                        OP.subtract, OP.mult)
            if g_sb is not None:
                nc.vector.tensor_mul(ln16[:rows], ln16[:rows], g_sb[:rows])
            if be_sb is not None:
                nc.vector.tensor_add(ln16[:rows], ln16[:rows], be_sb[:rows])
            return ln16

        from contextlib import contextmanager

        @contextmanager
        def _null_ctx():
            yield

        # ---- prologue: warmup, dec + enc-tb0 projections, LNs, transposes
        warmup(6, "a")

        dps = mpsum.tile([P, J], f32, tag="mps", name="dmm")
        for k in range(D // P):
            nc.tensor.matmul(dps[:U], dxT_sb[:, k, :], wdec_sb[:, k, :],
                             start=(k == 0), stop=(k == D // P - 1))
        decln = layer_norm_psum(dps, U, "d", b_dec_sb, g_dec_sb, be_dec_sb,
                                hot=True)

        eps_mm0 = mpsum.tile([P, J], f32, tag="mps", name="emm_0")
        for k in range(E // P):
            nc.tensor.matmul(eps_mm0[:], xT_sb[:, k, 0:P], wenc_sb[:, k, :],
                             start=(k == 0), stop=(k == E // P - 1))
        encln0 = layer_norm_psum(eps_mm0, P, "e0", b_enc_sb, g_enc_sb,
                                 be_enc_sb, hot=True)

        warmup(3, "b")

        for jb in range(J // P):
            nc.tensor.transpose(decPT[:, jb, :], decln[:U, jb * P:(jb + 1) * P],
                                ident[:U, :U])
        # decT is tiny: stage it to SBUF so the joint add has only one PSUM
        # operand (the BIR verifier rejects tensor_tensor with two)
        decT = const.tile([P, J // P, U], bf16)
        with tc.high_priority():
            nc.scalar.copy(decT[:], decPT[:])
        for jb in range(J // P):
            nc.tensor.transpose(encPT[:, jb, 0:P], encln0[:, jb * P:(jb + 1) * P],
                                ident[:])

        # ---- main loop: 32 supertiles x 512 rows ----
        KJ = J // P          # 4 contraction blocks
        TSUP = 512 // U      # 8 t values per supertile
        out_r = out[:].rearrange("(mm j p) v -> mm p j v", j=4, p=P)

        def supertile(mm):
            joint = jpool.tile([P, KJ, 512], bf16, tag="joint")
            jr = jrpool.tile([P, KJ, 512], bf16, tag="jr")
            jv = joint.rearrange("p k (t u) -> p k t u", u=U)
            t0 = mm * TSUP
            nq = 4 if mm == 0 else 2    # quarters for the first supertile
            step = 8 // nq
            for h in range(nq):
                tsl = slice(h * step, h * step + step)
                enc_b = encPT[:, :, t0 + h * step:t0 + (h + 1) * step, None] \
                    .to_broadcast((P, KJ, step, U))
                dec_b = decT[:, :, None, :].to_broadcast((P, KJ, step, U))
                nc.vector.tensor_tensor(jv[:, :, tsl], dec_b, enc_b, OP.add)
                sl = slice(h * step * U, (h + 1) * step * U)
                nc.scalar.activation(out=jr[:, :, sl], in_=joint[:, :, sl],
                                     func=AF.Relu)
            for j in range(4):
                stage = opool.tile([P, V], bf16, tag="stage", name=f"st_{mm}_{j}")
                pss = mpsum.tile([P, V], f32, tag="mps", name=f"ps_{mm}_{j}")
                for k in range(KJ):
                    for v in range(V // 512):
                        nc.tensor.matmul(
                            pss[:, v * 512:(v + 1) * 512],
                            jr[:, k, j * P:(j + 1) * P],
                            wout_sb[:, k, v * 512:(v + 1) * 512],
                            start=(k == 0), stop=(k == KJ - 1))
                last = (mm == MM_TILES - 1 and j == 3)
                if b_out_sb is not None:
                    nc.vector.tensor_add(stage[:, :512], pss[:, :512],
                                         b_out_sb[:, :512])
                    nc.scalar.tensor_add(stage[:, 512:], pss[:, 512:],
                                         b_out_sb[:, 512:])
                elif last:
                    # split the final eviction so DVE+ACT finish it in parallel
                    nc.vector.tensor_copy(stage[:, :512], pss[:, :512])
                    nc.scalar.copy(stage[:, 512:], pss[:, 512:])
                elif j % 2 == 0:
                    nc.vector.tensor_copy(stage[:], pss[:])
                else:
                    nc.scalar.copy(stage[:], pss[:])
                if last:
                    nc.sync.dma_start(out_r[mm, :, j, 0:512], stage[:, :512])
                    nc.sync.dma_start(out_r[mm, :, j, 512:], stage[:, 512:])
                else:
                    nc.sync.dma_start(out_r[mm, :, j], stage[:])

        supertile(0)

        # deferred enc tb1 projection: hides inside the main loop on the PE
        eps_mm1 = mpsum.tile([P, J], f32, tag="mps", name="emm_1")
        for k in range(E // P):
            nc.tensor.matmul(eps_mm1[:], xT_sb[:, k, P:2 * P], wenc_sb[:, k, :],
                             start=(k == 0), stop=(k == E // P - 1))
        encln1 = layer_norm_psum(eps_mm1, P, "e1", b_enc_sb, g_enc_sb, be_enc_sb)
        for jb in range(J // P):
            nc.tensor.transpose(encPT[:, jb, P:2 * P],
                                encln1[:, jb * P:(jb + 1) * P], ident[:])

        for mm in range(1, MM_TILES):
            supertile(mm)

    nc.compile()
    return nc


def kernel(**inputs):
    import ml_dtypes
    from concourse.bass_utils import run_bass_kernel_spmd

    bf = ml_dtypes.bfloat16
    enc = np.asarray(inputs["encoder_out"], dtype=np.float32)
    dec = np.asarray(inputs["decoder_out"], dtype=np.float32)
    named = {}
    for k_src, k_dst in [("b_enc", "b_enc"), ("g_enc", "g_enc"),
                         ("be_enc", "be_enc"), ("b_dec", "b_dec"),
                         ("g_dec", "g_dec"), ("be_dec", "be_dec"),
                         ("b_out", "b_out")]:
        named[k_dst] = np.ascontiguousarray(
            np.asarray(inputs[k_src], dtype=np.float32))
    for k_src, k_dst in [("W_enc", "w_enc"), ("W_dec", "w_dec"),
                         ("W_out", "w_out")]:
        named[k_dst] = np.ascontiguousarray(
            np.asarray(inputs[k_src], dtype=np.float32).astype(bf))

    flags = (
        bool(np.any(named["b_enc"])), not np.all(named["g_enc"] == 1.0),
        bool(np.any(named["be_enc"])),
        bool(np.any(named["b_dec"])), not np.all(named["g_dec"] == 1.0),
        bool(np.any(named["be_dec"])),
        bool(np.any(named["b_out"])),
    )
    if flags not in _CACHE:
        _CACHE[flags] = _build(*flags)
    nc = _CACHE[flags]

    tpc = T // (NCORES // B)      # t-rows per core
    in_maps = []
    for c in range(NCORES):
        b = c // (NCORES // B)
        t0 = (c % (NCORES // B)) * tpc
        in_maps.append({
            "enc_xT": np.ascontiguousarray(enc[b, t0:t0 + tpc].T.astype(bf)),
            "dec_xT": np.ascontiguousarray(dec[b].T.astype(bf)),
            **named,
        })

    res = run_bass_kernel_spmd(nc, in_maps, core_ids=list(range(NCORES)))
    full = np.concatenate(
        [np.asarray(res.results[c]["out"]).astype(np.float32)
         for c in range(NCORES)], axis=0)
    return full.reshape(B, T, U, V)
